# revision 31
# baseline (speedup 1.0000x reference)
"""Trainium2 Bass kernel for nn_MemoryBank_51135880626820 (scatter_memory).

Data-parallel over the query batch across 8 NeuronCores: the [32768, 256]
memory bank is replicated per core, each core handles 1024 query rows.

Per-core pipeline (v2):
  prep (chased by main loop):
    - load memory in 512-row groups (fp32), square (DVE) + 3D-reduce (gpsimd)
      for exact fp32 norms, sqrt/clamp/recip -> rinv
    - scale rows by rinv on the scalar engine, cast fp16, write nm_dram
    - DMA-transpose nm_dram into a resident d-major [2x128, 32768] fp16 bank
    - write rinv (fp32, partition-major image) to rinv_dram for refine gather
  per 128-query tile:
    - fp16 matmul (fp32 PSUM) over 2048-col chunks -> scalar evac to fp16 sim
      sector [128,16384] -> DVE max8/find_index8 per sector (2 sectors ->
      16 exact candidates with u16 indices)
    - one batched indirect DMA gathers 16 raw fp32 memory rows per query and
      a second gathers the 16 fp32 rinv values
    - exact fp32 re-dot (TTR) * rinv -> refined sims; top-8 + masked softmax
    - weighted sum on gpsimd (in-place over gathered rows), renormalize to
      the query's L2 norm

self-contained: hardcodes all shapes; builds and caches the Bass program on
first call.
"""

import os
import sys

DEBUG = os.environ.get("MB_DEBUG") == "1"

for _p in ("/opt/trn_rl_repo",):
    if _p not in sys.path:
        sys.path.insert(0, _p)

import numpy as np

import concourse.bass as bass
import concourse.mybir as mybir
import concourse.tile as tile
from concourse.bass import IndirectOffsetOnAxis

F32 = mybir.dt.float32
F16 = mybir.dt.float16
U16 = mybir.dt.uint16
U32 = mybir.dt.uint32

N_CORES = 8
B = 8192
B_LOC = B // N_CORES        # 1024
M = 32768
D = 256
K = 8
NQT = B_LOC // 128          # 8 query tiles per core
SEC = 8192                  # screening quarter-sector width (fp16 SBUF)
CH = 2048                   # PSUM chunk (4 banks)
GR = 512                    # prep group rows
NG = M // GR                # 64 prep groups
TPG = GR // 128             # 4 m-tiles per prep group
LP = 16                     # refined candidates
NEG = -1.0e30
SELF_MATCH = 0.9999
EPS = 1e-12


# --------------------------------------------------------------------------
# workarounds for this container's walrus build, which rejects more than one
# sync-wait per instruction ("Too many sync wait commands").
# --------------------------------------------------------------------------
def _install_patches():
    import json

    import bass_rust
    import concourse.bass_utils as _bu
    import concourse.bass2jax as _b2j
    import concourse.tile as tile_mod
    from concourse.tile import TileContext

    if getattr(_bu, "_mb_patched", False):
        return

    try:
        ScopedClock = tile_mod.ScopedClock
    except AttributeError:
        ScopedClock = bass_rust.ScopedClock

    def _patched_drain_and_barrier(self, tick_clock, wait_clock):
        nc = self.nc
        drain_inst = nc.sync.drain()
        wait_clock.add_sem_waits(
            drain_inst.ins, ScopedClock({None: tick_clock.global_clock})
        )
        si = drain_inst.ins.sync_info
        waits = list(si.on_wait) if si is not None and si.on_wait else []
        if len(waits) > 1:
            drain_inst.ins.sync_info = bass_rust.SyncInfo(
                on_wait=[waits[0]],
                on_update=list(si.on_update) if si.on_update else [],
            )
            for w in waits[1:]:
                nop = nc.sync.nop(nofuse=True, hint="tail_wait")
                nop.ins.sync_info = bass_rust.SyncInfo(on_wait=[w], on_update=[])
        nc.all_engine_barrier()
        assert self.sems is not None
        popped = nc._tile_sem_poison_stack.pop()
        assert popped is self._sem_poison
        nc.clear_and_free_semaphores(list(self.sems.allocated().values()))
        nc.all_engine_barrier()

    TileContext._drain_and_barrier = _patched_drain_and_barrier

    def split_multiwaits(bir_json):
        m = json.loads(bir_json)
        changed = False
        for fn in m.get("functions", []):
            for bb in fn.get("blocks", []):
                insts = bb.get("instructions", [])
                out = []
                for ins in insts:
                    si = ins.get("sync_info") or {}
                    waits = si.get("on_wait") or []
                    if len(waits) > 1:
                        changed = True
                        for kk, w in enumerate(waits[:-1]):
                            out.append({
                                "debug": ins.get("debug", 0),
                                "engine": ins["engine"],
                                "ins": [],
                                "name": f"{ins['name']}-w{kk}",
                                "opcode": "NoOp",
                                "outs": [],
                                "sync_info": {"on_update": [], "on_wait": [w]},
                                "text_hint": "split_wait",
                            })
                        si = dict(si)
                        si["on_wait"] = [waits[-1]]
                        ins = dict(ins)
                        ins["sync_info"] = si
                    out.append(ins)
                bb["instructions"] = out
        if not changed:
            return bir_json
        return json.dumps(m).encode()

    _orig_compile = _bu.compile_bir_kernel

    def _patched_compile(bir_json, tmpdir, neff_name="file.neff"):
        if isinstance(bir_json, str):
            bir_json = bir_json.encode()
        return _orig_compile(split_multiwaits(bir_json), tmpdir, neff_name)

    _bu.compile_bir_kernel = _patched_compile
    _b2j.compile_bir_kernel = _patched_compile
    _bu._mb_patched = True


# --------------------------------------------------------------------------
# per-core Bass program
# --------------------------------------------------------------------------
def _build():
    nc = bass.Bass("TRN2", target_bir_lowering=False, debug=False)
    q_in = nc.dram_tensor("q", [B_LOC, D], F32, kind="ExternalInput")
    mem_in = nc.dram_tensor("mem", [M, D], F32, kind="ExternalInput")
    out = nc.dram_tensor("out", [B_LOC, D], F32, kind="ExternalOutput")
    nm_dram = nc.dram_tensor("nm_dram", [M, D], F16)
    # per-row 1/max(||m||,eps), row-major (written groupwise during prep)
    rinv_dram = nc.dram_tensor("rinv_dram", [M, 1], F32)
    if DEBUG:
        dbg = {
            "simA": nc.dram_tensor("dbg_simA", [128, SEC], F16, kind="ExternalOutput"),
            "candv": nc.dram_tensor("dbg_candv", [128, LP], F16, kind="ExternalOutput"),
            "candi": nc.dram_tensor("dbg_candi", [128, LP], U16, kind="ExternalOutput"),
            "rows": nc.dram_tensor("dbg_rows", [128, LP], U32, kind="ExternalOutput"),
            "roff": nc.dram_tensor("dbg_roff", [128, LP], U32, kind="ExternalOutput"),
            "RV": nc.dram_tensor("dbg_RV", [128, LP], F32, kind="ExternalOutput"),
            "G": nc.dram_tensor("dbg_G", [128, LP * D], F32, kind="ExternalOutput"),
            "dots": nc.dram_tensor("dbg_dots", [128, LP], F32, kind="ExternalOutput"),
            "refined": nc.dram_tensor("dbg_refined", [128, LP], F32, kind="ExternalOutput"),
            "wts": nc.dram_tensor("dbg_wts", [128, LP], F32, kind="ExternalOutput"),
            "acc": nc.dram_tensor("dbg_acc", [128, D], F32, kind="ExternalOutput"),
            "nmT_a": nc.dram_tensor("dbg_nmT_a", [128, 2048], F16, kind="ExternalOutput"),
            "qT_a": nc.dram_tensor("dbg_qT_a", [128, 128], F16, kind="ExternalOutput"),
        }

    with tile.TileContext(nc) as tc:
        with (
            tc.tile_pool(name="res", bufs=1) as res,
            tc.tile_pool(name="prep2", bufs=2) as prep2,
            tc.tile_pool(name="prep1", bufs=1) as prep1,
            tc.tile_pool(name="small", bufs=2) as small,
            tc.tile_pool(name="secp", bufs=2) as secp,
            tc.tile_pool(name="psum", bufs=2, space="PSUM") as psp,
        ):
            nmT_a = res.tile([128, M], F16, tag="nmT_a")
            nmT_b = res.tile([128, M], F16, tag="nmT_b")
            G = res.tile([128, LP * D], F32, tag="G")
            # fp16 tiebreak row (-6e-5 * slot) so candidate values are
            # distinct before the is_equal index extraction
            tb32 = res.tile([128, 2 * LP], F16, tag="tb32")
            tbu = res.tile([128, 2 * LP], U16, tag="tbu")
            nc.gpsimd.iota(tbu[:], pattern=[[1, 2 * LP]], base=0,
                           channel_multiplier=0)
            tbf = res.tile([128, 2 * LP], F32, tag="tbf")
            nc.vector.tensor_copy(tbf[:], tbu[:])
            nc.vector.tensor_scalar(
                out=tb32[:], in0=tbf[:], scalar1=-6e-5, scalar2=None,
                op0=mybir.AluOpType.mult)

            # ---------------- prep: one group of 512 memory rows ----------
            # rows-per-partition layout: partition p holds rows
            # r0 + p*TPG .. r0 + p*TPG + TPG-1 (contiguous), so the group
            # load / nm write / rinv write are each ONE dma with big
            # contiguous per-partition runs, and rinv_dram is row-major.
            def prep_group(g):
                r0 = g * GR
                ml = prep2.tile([128, TPG * D], F32, tag="mload")
                nc.sync.dma_start(
                    ml[:],
                    mem_in[r0 : r0 + GR, :].rearrange(
                        "(p e) d -> p (e d)", p=128),
                )
                n2 = prep1.tile([128, TPG], F32, tag="n2")
                msq = prep1.tile([128, TPG * D], F32, tag="msq")
                nc.gpsimd.tensor_tensor(
                    out=msq[:], in0=ml[:], in1=ml[:], op=mybir.AluOpType.mult)
                nc.vector.tensor_reduce(
                    out=n2[:], in_=msq[:].rearrange("p (t d) -> p t d", t=TPG),
                    axis=mybir.AxisListType.X, op=mybir.AluOpType.add)
                nrm = prep1.tile([128, TPG], F32, tag="nrm")
                nc.scalar.activation(
                    nrm[:], n2[:], mybir.ActivationFunctionType.Sqrt
                )
                nc.vector.tensor_scalar_max(nrm[:], nrm[:], EPS)
                rin = prep1.tile([128, TPG], F32, tag="rin")
                nc.vector.reciprocal(rin[:], nrm[:])
                nc.sync.dma_start(
                    rinv_dram[r0 : r0 + GR, :].rearrange(
                        "(p e) one -> p (e one)", p=128),
                    rin[:],
                )
                nmb = prep2.tile([128, TPG * D], F16, tag="nmb")
                for t in range(TPG):
                    nc.scalar.activation(
                        nmb[:, t * D : (t + 1) * D],
                        ml[:, t * D : (t + 1) * D],
                        mybir.ActivationFunctionType.Copy,
                        scale=rin[:, t : t + 1],
                    )
                nc.scalar.dma_start(
                    nm_dram[r0 : r0 + GR, :].rearrange(
                        "(p e) d -> p (e d)", p=128),
                    nmb[:],
                )
                nc.sync.dma_start(
                    nmT_a[:, r0 : r0 + GR],
                    nm_dram[r0 : r0 + GR, 0:128],
                    transpose=True,
                )
                nc.sync.dma_start(
                    nmT_b[:, r0 : r0 + GR],
                    nm_dram[r0 : r0 + GR, 128:256],
                    transpose=True,
                )

            # ---------------- main: one tile of 128 queries ---------------
            def query_prep(qt):
                q0 = qt * 128
                qf = small.tile([128, D], F32, tag="qf")
                nc.sync.dma_start(qf[:], q_in[q0 : q0 + 128, :])
                qtr = prep1.tile([128, D], F32, tag="trash")
                qn2 = small.tile([128, 1], F32, tag="qn2")
                nc.vector.tensor_tensor(
                    out=qtr[:], in0=qf[:], in1=qf[:], op=mybir.AluOpType.mult)
                nc.vector.tensor_reduce(
                    out=qn2[:], in_=qtr[:], axis=mybir.AxisListType.X,
                    op=mybir.AluOpType.add)
                qnorm = small.tile([128, 1], F32, tag="qnorm")
                nc.scalar.activation(
                    qnorm[:], qn2[:], mybir.ActivationFunctionType.Sqrt
                )
                qcl = small.tile([128, 1], F32, tag="qcl")
                nc.vector.tensor_scalar_max(qcl[:], qnorm[:], EPS)
                qrin = small.tile([128, 1], F32, tag="qrin")
                nc.vector.reciprocal(qrin[:], qcl[:])
                nqf = small.tile([128, D], F32, tag="nqf")
                nc.scalar.activation(
                    nqf[:], qf[:], mybir.ActivationFunctionType.Copy,
                    scale=qrin[:],
                )
                nqh = small.tile([128, D], F16, tag="nqh")
                nc.scalar.activation(
                    nqh[:], qf[:], mybir.ActivationFunctionType.Copy,
                    scale=qrin[:],
                )
                qT_a = small.tile([128, 128], F16, tag="qT_a")
                qT_b = small.tile([128, 128], F16, tag="qT_b")
                nc.sync.dma_start(qT_a[:], nqh[:, 0:128], transpose=True)
                nc.sync.dma_start(qT_b[:], nqh[:, 128:256], transpose=True)
                return qf, qnorm, nqf, qT_a, qT_b

            def tile_chunk(c, qT_a, qT_b, sim):
                """matmul chunk c (2048 cols) + evac into sim quarter slot."""
                m0 = c * CH
                ps = psp.tile([128, CH], F32, tag="ps")
                for b_ in range(CH // 512):
                    nc.tensor.matmul(
                        ps[:, b_ * 512 : (b_ + 1) * 512], qT_a[:],
                        nmT_a[:, m0 + b_ * 512 : m0 + (b_ + 1) * 512],
                        start=True, stop=False)
                for b_ in range(CH // 512):
                    nc.tensor.matmul(
                        ps[:, b_ * 512 : (b_ + 1) * 512], qT_b[:],
                        nmT_b[:, m0 + b_ * 512 : m0 + (b_ + 1) * 512],
                        start=False, stop=True)
                s0 = (c * CH) % SEC
                nc.scalar.copy(sim[:, s0 : s0 + CH], ps[:])

            def screen_quarter(h, candv, candi, sim):
                nc.vector.max(out=candv[:, h * 8 : (h + 1) * 8], in_=sim[:])
                nc.vector.max_index(
                    out=candi[:],
                    in_max=candv[:, h * 8 : (h + 1) * 8],
                    in_values=sim[:],
                )

            def refine_gather(qt, candv, candif):
                # prune 32 quarter-candidates to top-16 and extract their
                # row ids (baseline-proven is_equal pattern, fp16 tiebreak
                # keeps values distinct)
                cvt = small.tile([128, 2 * LP], F32, tag="cvt")
                nc.vector.tensor_tensor(
                    out=cvt[:], in0=candv[:], in1=tb32[:],
                    op=mybir.AluOpType.add)
                pv1 = small.tile([128, 8], F32, tag="pv1")
                nc.vector.max(out=pv1[:], in_=cvt[:])
                cv2 = small.tile([128, 2 * LP], F32, tag="cv2")
                nc.vector.match_replace(
                    out=cv2[:], in_to_replace=pv1[:], in_values=cvt[:],
                    imm_value=NEG)
                pv2 = small.tile([128, 8], F32, tag="pv2")
                nc.vector.max(out=pv2[:], in_=cv2[:])
                pidx = small.tile([128, LP], F32, tag="pidx")
                msk = small.tile([128, 2 * LP], F32, tag="msk")
                mprod = small.tile([128, 2 * LP], F32, tag="mprod")
                for kk in range(LP):
                    pv = pv1 if kk < 8 else pv2
                    srcv = cvt if kk < 8 else cv2
                    nc.vector.tensor_scalar(
                        out=msk[:], in0=srcv[:],
                        scalar1=pv[:, kk % 8 : kk % 8 + 1], scalar2=None,
                        op0=mybir.AluOpType.is_equal)
                    nc.vector.tensor_tensor(
                        out=mprod[:], in0=msk[:], in1=candif[:],
                        op=mybir.AluOpType.mult)
                    nc.vector.tensor_reduce(
                        out=pidx[:, kk : kk + 1], in_=mprod[:],
                        axis=mybir.AxisListType.X, op=mybir.AluOpType.add)
                rows = small.tile([128, LP], U32, tag="rows")
                nc.vector.tensor_copy(rows[:], pidx[:])
                if DEBUG and qt == 0:
                    nc.sync.dma_start(dbg["rows"][:, :], rows[:])
                    nc.sync.dma_start(dbg["roff"][:, :], rows[:])
                for k in range(LP):
                    nc.gpsimd.indirect_dma_start(
                        out=G[:, k * D : (k + 1) * D],
                        out_offset=None,
                        in_=mem_in[:],
                        in_offset=IndirectOffsetOnAxis(
                            ap=rows[:, k : k + 1], axis=0))
                RV = small.tile([128, LP], F32, tag="RV")
                for k in range(LP):
                    nc.gpsimd.indirect_dma_start(
                        out=RV[:, k : k + 1],
                        out_offset=None,
                        in_=rinv_dram[:],
                        in_offset=IndirectOffsetOnAxis(
                            ap=rows[:, k : k + 1], axis=0))

                return RV

            def refine_finish(qt, qnorm, nqf, RV):
                q0 = qt * 128
                if DEBUG and qt == 0:
                    nc.sync.dma_start(dbg["G"][:, :], G[:])
                    nc.sync.dma_start(dbg["RV"][:, :], RV[:])
                # dots: gpsimd does the (G * nq) products in quarters, DVE
                # only the 3D reduces
                dots = small.tile([128, LP], F32, tag="dots")
                QW = 2  # candidates per sub-batch
                gq = prep1.tile([128, QW * D], F32, tag="gq")
                nq3 = nqf[:].rearrange("p (o d) -> p o d", o=1).to_broadcast(
                    [128, QW, D])
                for h in range(LP // QW):
                    nc.gpsimd.tensor_tensor(
                        out=gq[:].rearrange("p (k d) -> p k d", k=QW),
                        in0=G[:, h * QW * D : (h + 1) * QW * D].rearrange(
                            "p (k d) -> p k d", k=QW),
                        in1=nq3, op=mybir.AluOpType.mult)
                    nc.vector.tensor_reduce(
                        out=dots[:, h * QW : (h + 1) * QW],
                        in_=gq[:].rearrange("p (k d) -> p k d", k=QW),
                        axis=mybir.AxisListType.X, op=mybir.AluOpType.add)
                refined = small.tile([128, LP], F32, tag="refined")
                nc.vector.tensor_tensor(
                    out=refined[:], in0=dots[:], in1=RV[:],
                    op=mybir.AluOpType.mult)

                if DEBUG and qt == 0:
                    nc.sync.dma_start(dbg["dots"][:, :], dots[:])
                # self-match mask (exact, fp32)
                selfm = small.tile([128, LP], F32, tag="selfm")
                nc.vector.tensor_scalar(
                    out=selfm[:], in0=refined[:], scalar1=SELF_MATCH,
                    scalar2=NEG, op0=mybir.AluOpType.is_ge,
                    op1=mybir.AluOpType.mult)
                nc.vector.tensor_add(refined[:], refined[:], selfm[:])

                if DEBUG and qt == 0:
                    nc.sync.dma_start(dbg["refined"][:, :], refined[:])
                top8 = small.tile([128, 8], F32, tag="top8")
                nc.vector.max(out=top8[:], in_=refined[:])
                wmask = small.tile([128, LP], F32, tag="wmask")
                nc.vector.tensor_scalar(
                    out=wmask[:], in0=refined[:], scalar1=top8[:, 7:8],
                    scalar2=None, op0=mybir.AluOpType.is_ge)
                shift = small.tile([128, LP], F32, tag="shift")
                nc.vector.tensor_scalar(
                    out=shift[:], in0=refined[:], scalar1=top8[:, 0:1],
                    scalar2=None, op0=mybir.AluOpType.subtract)
                expv = small.tile([128, LP], F32, tag="expv")
                nc.scalar.activation(
                    expv[:], shift[:], mybir.ActivationFunctionType.Exp)
                wts = small.tile([128, LP], F32, tag="wts")
                nc.vector.tensor_tensor(
                    out=wts[:], in0=expv[:], in1=wmask[:],
                    op=mybir.AluOpType.mult)
                zsum = small.tile([128, 1], F32, tag="zsum")
                nc.vector.tensor_reduce(
                    out=zsum[:], in_=wts[:], axis=mybir.AxisListType.X,
                    op=mybir.AluOpType.add)
                zrin = small.tile([128, 1], F32, tag="zrin")
                nc.vector.reciprocal(zrin[:], zsum[:])
                nc.vector.tensor_scalar(
                    out=wts[:], in0=wts[:], scalar1=zrin[:, 0:1],
                    scalar2=None, op0=mybir.AluOpType.mult)

                if DEBUG and qt == 0:
                    nc.sync.dma_start(dbg["wts"][:, :], wts[:])
                # weighted sum on gpsimd: G *= w (in place), then reduce over k
                g3 = G[:].rearrange("p (k d) -> p k d", k=LP)
                w3 = wts[:].rearrange("p (k o) -> p k o", k=LP).to_broadcast(
                    [128, LP, D])
                nc.gpsimd.tensor_tensor(
                    out=g3, in0=g3, in1=w3, op=mybir.AluOpType.mult)
                acc = prep1.tile([128, D], F32, tag="acc")
                nc.vector.tensor_reduce(
                    out=acc[:],
                    in_=G[:].rearrange("p (k d) -> p d k", k=LP),
                    axis=mybir.AxisListType.X,
                    op=mybir.AluOpType.add)

                if DEBUG and qt == 0:
                    nc.sync.dma_start(dbg["acc"][:, :], acc[:])
                # renormalize to ||q||
                atr = prep1.tile([128, D], F32, tag="trash")
                an2 = small.tile([128, 1], F32, tag="an2")
                nc.vector.tensor_tensor(
                    out=atr[:], in0=acc[:], in1=acc[:], op=mybir.AluOpType.mult)
                nc.vector.tensor_reduce(
                    out=an2[:], in_=atr[:], axis=mybir.AxisListType.X,
                    op=mybir.AluOpType.add)
                an = small.tile([128, 1], F32, tag="an")
                nc.scalar.activation(
                    an[:], an2[:], mybir.ActivationFunctionType.Sqrt)
                nc.vector.tensor_scalar_max(an[:], an[:], EPS)
                arin = small.tile([128, 1], F32, tag="arin")
                nc.vector.reciprocal(arin[:], an[:])
                scl = small.tile([128, 1], F32, tag="scl")
                nc.vector.tensor_tensor(
                    out=scl[:], in0=arin[:], in1=qnorm[:],
                    op=mybir.AluOpType.mult)
                ot = prep1.tile([128, D], F32, tag="ot")
                nc.scalar.activation(
                    ot[:], acc[:], mybir.ActivationFunctionType.Copy,
                    scale=scl[:])
                nc.sync.dma_start(out[q0 : q0 + 128, :], ot[:])

            # ---------------- emission: prep chased by tile 0 -------------
            GPC = CH // GR  # groups per chunk = 4

            # software-pipelined emission: refine of tile t-1 is emitted
            # after the screens of tile t, so the scalar engine's evacs of
            # tile t are not queued behind tile t-1's refine-phase
            # activations, and the DVE alternates screens / refine work.
            # Prep groups are interleaved with tile-0 chunks so the bank
            # build is chased by the first tile.
            GPC = CH // GR  # groups per chunk

            NQRT = M // SEC  # quarters per tile
            CPQ = SEC // CH  # chunks per quarter

            def cand_convert(h, candv, candif, candi):
                # candidate col ids -> global f32 row ids
                nc.vector.tensor_copy(
                    candif[:, h * 8 : (h + 1) * 8], candi[:])
                if h:
                    nc.vector.tensor_scalar(
                        out=candif[:, h * 8 : (h + 1) * 8],
                        in0=candif[:, h * 8 : (h + 1) * 8],
                        scalar1=float(h * SEC), scalar2=None,
                        op0=mybir.AluOpType.add)

            # ---- phase 1: pure prep. Interleaving tiles with the bank
            # build measured SLOWER (the prep chain's scalar/DVE hops queue
            # behind tile evacs/screens in engine program order); with clean
            # queues the 64 groups pipeline at full depth.
            for g in range(NG):
                prep_group(g)
            pending = None

            # ---- phase 2: tiles 0..7, refine of tile t-1 interleaved after
            # quarter 1 so its scalar/gpsimd work overlaps the screens
            gathered = None
            for qt in range(0, NQT):
                qf, qnorm, nqf, qT_a, qT_b = query_prep(qt)
                candv = small.tile([128, 2 * LP], F16, tag="candv")
                candif = small.tile([128, 2 * LP], F32, tag="candif")
                for h in range(NQRT):
                    sim = secp.tile([128, SEC], F16, tag="sim")
                    for c in range(h * CPQ, (h + 1) * CPQ):
                        tile_chunk(c, qT_a, qT_b, sim)
                    candi = small.tile([128, 8], U16, tag="candi")
                    screen_quarter(h, candv, candi, sim)
                    cand_convert(h, candv, candif, candi)
                    if h == 1 and pending is not None:
                        pq, pqn, pnq, pcv, pcf = pending
                        rv = refine_gather(pq, pcv, pcf)
                        gathered = (pq, pqn, pnq, rv)
                        pending = None
                    if h == 3 and gathered is not None:
                        refine_finish(*gathered)
                        gathered = None
                pending = (qt, qnorm, nqf, candv, candif)
            pq, pqn, pnq, pcv, pcf = pending
            rv = refine_gather(pq, pcv, pcf)
            refine_finish(pq, pqn, pnq, rv)

    return nc


_CACHED_NC = None


def _get_nc():
    global _CACHED_NC
    if _CACHED_NC is None:
        _install_patches()
        _CACHED_NC = _build()
    return _CACHED_NC


def kernel(query, memory, k):
    query = np.ascontiguousarray(np.asarray(query, dtype=np.float32))
    memory = np.ascontiguousarray(np.asarray(memory, dtype=np.float32))
    k_val = int(np.asarray(k))
    assert query.shape == (B, D) and memory.shape == (M, D), (query.shape, memory.shape)
    assert k_val == K, f"kernel compiled for k={K}, got {k_val}"

    from concourse.bass_utils import run_bass_kernel_spmd

    nc = _get_nc()
    in_maps = [
        {"q": query[i * B_LOC : (i + 1) * B_LOC], "mem": memory}
        for i in range(N_CORES)
    ]
    res = run_bass_kernel_spmd(nc, in_maps, list(range(N_CORES)))
    return np.concatenate([res.results[i]["out"] for i in range(N_CORES)], axis=0)


# revision 32
# speedup vs baseline: 1.1282x; 1.1282x over previous
"""Trainium2 Bass kernel for nn_MemoryBank_51135880626820 (scatter_memory).

Data-parallel over the query batch across 8 NeuronCores: the [32768, 256]
memory bank is replicated per core, each core handles 1024 query rows.

Per-core pipeline (v2):
  prep (chased by main loop):
    - load memory in 512-row groups (fp32), square (DVE) + 3D-reduce (gpsimd)
      for exact fp32 norms, sqrt/clamp/recip -> rinv
    - scale rows by rinv on the scalar engine, cast fp16, write nm_dram
    - DMA-transpose nm_dram into a resident d-major [2x128, 32768] fp16 bank
    - write rinv (fp32, partition-major image) to rinv_dram for refine gather
  per 128-query tile:
    - fp16 matmul (fp32 PSUM) over 2048-col chunks -> scalar evac to fp16 sim
      sector [128,16384] -> DVE max8/find_index8 per sector (2 sectors ->
      16 exact candidates with u16 indices)
    - one batched indirect DMA gathers 16 raw fp32 memory rows per query and
      a second gathers the 16 fp32 rinv values
    - exact fp32 re-dot (TTR) * rinv -> refined sims; top-8 + masked softmax
    - weighted sum on gpsimd (in-place over gathered rows), renormalize to
      the query's L2 norm

self-contained: hardcodes all shapes; builds and caches the Bass program on
first call.
"""

import os
import sys

DEBUG = os.environ.get("MB_DEBUG") == "1"

for _p in ("/opt/trn_rl_repo",):
    if _p not in sys.path:
        sys.path.insert(0, _p)

import numpy as np

import concourse.bass as bass
import concourse.mybir as mybir
import concourse.tile as tile
from concourse.bass import IndirectOffsetOnAxis

F32 = mybir.dt.float32
F16 = mybir.dt.float16
U16 = mybir.dt.uint16
U32 = mybir.dt.uint32

N_CORES = 8
B = 8192
B_LOC = B // N_CORES        # 1024
M = 32768
D = 256
K = 8
NQT = B_LOC // 128          # 8 query tiles per core
SEC = 8192                  # screening quarter-sector width (fp16 SBUF)
CH = 2048                   # PSUM chunk (4 banks)
GR = 512                    # prep group rows
NG = M // GR                # 64 prep groups
TPG = GR // 128             # 4 m-tiles per prep group
NCAND = 32                  # screened candidates (4 quarters x top-8)
LP = 12                     # refined candidates (pruned from NCAND)
NEG = -1.0e30
SELF_MATCH = 0.9999
EPS = 1e-12


# --------------------------------------------------------------------------
# workarounds for this container's walrus build, which rejects more than one
# sync-wait per instruction ("Too many sync wait commands").
# --------------------------------------------------------------------------
def _install_patches():
    import json

    import bass_rust
    import concourse.bass_utils as _bu
    import concourse.bass2jax as _b2j
    import concourse.tile as tile_mod
    from concourse.tile import TileContext

    if getattr(_bu, "_mb_patched", False):
        return

    try:
        ScopedClock = tile_mod.ScopedClock
    except AttributeError:
        ScopedClock = bass_rust.ScopedClock

    def _patched_drain_and_barrier(self, tick_clock, wait_clock):
        nc = self.nc
        drain_inst = nc.sync.drain()
        wait_clock.add_sem_waits(
            drain_inst.ins, ScopedClock({None: tick_clock.global_clock})
        )
        si = drain_inst.ins.sync_info
        waits = list(si.on_wait) if si is not None and si.on_wait else []
        if len(waits) > 1:
            drain_inst.ins.sync_info = bass_rust.SyncInfo(
                on_wait=[waits[0]],
                on_update=list(si.on_update) if si.on_update else [],
            )
            for w in waits[1:]:
                nop = nc.sync.nop(nofuse=True, hint="tail_wait")
                nop.ins.sync_info = bass_rust.SyncInfo(on_wait=[w], on_update=[])
        nc.all_engine_barrier()
        assert self.sems is not None
        popped = nc._tile_sem_poison_stack.pop()
        assert popped is self._sem_poison
        nc.clear_and_free_semaphores(list(self.sems.allocated().values()))
        nc.all_engine_barrier()

    TileContext._drain_and_barrier = _patched_drain_and_barrier

    def split_multiwaits(bir_json):
        m = json.loads(bir_json)
        changed = False
        for fn in m.get("functions", []):
            for bb in fn.get("blocks", []):
                insts = bb.get("instructions", [])
                out = []
                for ins in insts:
                    si = ins.get("sync_info") or {}
                    waits = si.get("on_wait") or []
                    if len(waits) > 1:
                        changed = True
                        for kk, w in enumerate(waits[:-1]):
                            out.append({
                                "debug": ins.get("debug", 0),
                                "engine": ins["engine"],
                                "ins": [],
                                "name": f"{ins['name']}-w{kk}",
                                "opcode": "NoOp",
                                "outs": [],
                                "sync_info": {"on_update": [], "on_wait": [w]},
                                "text_hint": "split_wait",
                            })
                        si = dict(si)
                        si["on_wait"] = [waits[-1]]
                        ins = dict(ins)
                        ins["sync_info"] = si
                    out.append(ins)
                bb["instructions"] = out
        if not changed:
            return bir_json
        return json.dumps(m).encode()

    _orig_compile = _bu.compile_bir_kernel

    def _patched_compile(bir_json, tmpdir, neff_name="file.neff"):
        if isinstance(bir_json, str):
            bir_json = bir_json.encode()
        return _orig_compile(split_multiwaits(bir_json), tmpdir, neff_name)

    _bu.compile_bir_kernel = _patched_compile
    _b2j.compile_bir_kernel = _patched_compile
    _bu._mb_patched = True


# --------------------------------------------------------------------------
# per-core Bass program
# --------------------------------------------------------------------------
def _build():
    nc = bass.Bass("TRN2", target_bir_lowering=False, debug=False)
    q_in = nc.dram_tensor("q", [B_LOC, D], F32, kind="ExternalInput")
    mem_in = nc.dram_tensor("mem", [M, D], F32, kind="ExternalInput")
    out = nc.dram_tensor("out", [B_LOC, D], F32, kind="ExternalOutput")
    nm_dram = nc.dram_tensor("nm_dram", [M, D], F16)
    # per-row 1/max(||m||,eps), row-major (written groupwise during prep)
    rinv_dram = nc.dram_tensor("rinv_dram", [M, 1], F32)
    if DEBUG:
        dbg = {
            "simA": nc.dram_tensor("dbg_simA", [128, SEC], F16, kind="ExternalOutput"),
            "candv": nc.dram_tensor("dbg_candv", [128, LP], F16, kind="ExternalOutput"),
            "candi": nc.dram_tensor("dbg_candi", [128, LP], U16, kind="ExternalOutput"),
            "rows": nc.dram_tensor("dbg_rows", [128, LP], U32, kind="ExternalOutput"),
            "roff": nc.dram_tensor("dbg_roff", [128, LP], U32, kind="ExternalOutput"),
            "RV": nc.dram_tensor("dbg_RV", [128, LP], F32, kind="ExternalOutput"),
            "G": nc.dram_tensor("dbg_G", [128, LP * D], F32, kind="ExternalOutput"),
            "dots": nc.dram_tensor("dbg_dots", [128, LP], F32, kind="ExternalOutput"),
            "refined": nc.dram_tensor("dbg_refined", [128, LP], F32, kind="ExternalOutput"),
            "wts": nc.dram_tensor("dbg_wts", [128, LP], F32, kind="ExternalOutput"),
            "acc": nc.dram_tensor("dbg_acc", [128, D], F32, kind="ExternalOutput"),
            "nmT_a": nc.dram_tensor("dbg_nmT_a", [128, 2048], F16, kind="ExternalOutput"),
            "qT_a": nc.dram_tensor("dbg_qT_a", [128, 128], F16, kind="ExternalOutput"),
        }

    with tile.TileContext(nc) as tc:
        with (
            tc.tile_pool(name="res", bufs=1) as res,
            tc.tile_pool(name="prep2", bufs=2) as prep2,
            tc.tile_pool(name="prep1", bufs=1) as prep1,
            tc.tile_pool(name="small", bufs=2) as small,
            tc.tile_pool(name="secp", bufs=2) as secp,
            tc.tile_pool(name="psum", bufs=2, space="PSUM") as psp,
        ):
            nmT_a = res.tile([128, M], F16, tag="nmT_a")
            nmT_b = res.tile([128, M], F16, tag="nmT_b")
            G = res.tile([128, LP * D], F32, tag="G")
            # fp16 tiebreak row (-6e-5 * slot) so candidate values are
            # distinct before the is_equal index extraction
            tb32 = res.tile([128, NCAND], F16, tag="tb32")
            tbu = res.tile([128, NCAND], U16, tag="tbu")
            nc.gpsimd.iota(tbu[:], pattern=[[1, NCAND]], base=0,
                           channel_multiplier=0)
            tbf = res.tile([128, NCAND], F32, tag="tbf")
            nc.vector.tensor_copy(tbf[:], tbu[:])
            nc.vector.tensor_scalar(
                out=tb32[:], in0=tbf[:], scalar1=-6e-5, scalar2=None,
                op0=mybir.AluOpType.mult)

            # ---------------- prep: one group of 512 memory rows ----------
            # rows-per-partition layout: partition p holds rows
            # r0 + p*TPG .. r0 + p*TPG + TPG-1 (contiguous), so the group
            # load / nm write / rinv write are each ONE dma with big
            # contiguous per-partition runs, and rinv_dram is row-major.
            def prep_group(g):
                r0 = g * GR
                ml = prep2.tile([128, TPG * D], F32, tag="mload")
                nc.sync.dma_start(
                    ml[:],
                    mem_in[r0 : r0 + GR, :].rearrange(
                        "(p e) d -> p (e d)", p=128),
                )
                n2 = prep1.tile([128, TPG], F32, tag="n2")
                msq = prep1.tile([128, TPG * D], F32, tag="msq")
                nc.gpsimd.tensor_tensor(
                    out=msq[:], in0=ml[:], in1=ml[:], op=mybir.AluOpType.mult)
                nc.vector.tensor_reduce(
                    out=n2[:], in_=msq[:].rearrange("p (t d) -> p t d", t=TPG),
                    axis=mybir.AxisListType.X, op=mybir.AluOpType.add)
                nrm = prep1.tile([128, TPG], F32, tag="nrm")
                nc.scalar.activation(
                    nrm[:], n2[:], mybir.ActivationFunctionType.Sqrt
                )
                nc.vector.tensor_scalar_max(nrm[:], nrm[:], EPS)
                rin = prep1.tile([128, TPG], F32, tag="rin")
                nc.vector.reciprocal(rin[:], nrm[:])
                nc.sync.dma_start(
                    rinv_dram[r0 : r0 + GR, :].rearrange(
                        "(p e) one -> p (e one)", p=128),
                    rin[:],
                )
                nmb = prep2.tile([128, TPG * D], F16, tag="nmb")
                for t in range(TPG):
                    nc.scalar.activation(
                        nmb[:, t * D : (t + 1) * D],
                        ml[:, t * D : (t + 1) * D],
                        mybir.ActivationFunctionType.Copy,
                        scale=rin[:, t : t + 1],
                    )
                nc.scalar.dma_start(
                    nm_dram[r0 : r0 + GR, :].rearrange(
                        "(p e) d -> p (e d)", p=128),
                    nmb[:],
                )
                nc.sync.dma_start(
                    nmT_a[:, r0 : r0 + GR],
                    nm_dram[r0 : r0 + GR, 0:128],
                    transpose=True,
                )
                nc.sync.dma_start(
                    nmT_b[:, r0 : r0 + GR],
                    nm_dram[r0 : r0 + GR, 128:256],
                    transpose=True,
                )

            # ---------------- main: one tile of 128 queries ---------------
            def query_prep(qt):
                q0 = qt * 128
                qf = small.tile([128, D], F32, tag="qf")
                nc.sync.dma_start(qf[:], q_in[q0 : q0 + 128, :])
                qtr = prep1.tile([128, D], F32, tag="trash")
                qn2 = small.tile([128, 1], F32, tag="qn2")
                nc.vector.tensor_tensor(
                    out=qtr[:], in0=qf[:], in1=qf[:], op=mybir.AluOpType.mult)
                nc.vector.tensor_reduce(
                    out=qn2[:], in_=qtr[:], axis=mybir.AxisListType.X,
                    op=mybir.AluOpType.add)
                qnorm = small.tile([128, 1], F32, tag="qnorm")
                nc.scalar.activation(
                    qnorm[:], qn2[:], mybir.ActivationFunctionType.Sqrt
                )
                qcl = small.tile([128, 1], F32, tag="qcl")
                nc.vector.tensor_scalar_max(qcl[:], qnorm[:], EPS)
                qrin = small.tile([128, 1], F32, tag="qrin")
                nc.vector.reciprocal(qrin[:], qcl[:])
                nqf = small.tile([128, D], F32, tag="nqf")
                nc.scalar.activation(
                    nqf[:], qf[:], mybir.ActivationFunctionType.Copy,
                    scale=qrin[:],
                )
                nqh = small.tile([128, D], F16, tag="nqh")
                nc.scalar.activation(
                    nqh[:], qf[:], mybir.ActivationFunctionType.Copy,
                    scale=qrin[:],
                )
                qT_a = small.tile([128, 128], F16, tag="qT_a")
                qT_b = small.tile([128, 128], F16, tag="qT_b")
                nc.sync.dma_start(qT_a[:], nqh[:, 0:128], transpose=True)
                nc.sync.dma_start(qT_b[:], nqh[:, 128:256], transpose=True)
                return qf, qnorm, nqf, qT_a, qT_b

            def tile_chunk(c, qT_a, qT_b, sim):
                """matmul chunk c (2048 cols) + evac into sim quarter slot."""
                m0 = c * CH
                ps = psp.tile([128, CH], F32, tag="ps")
                for b_ in range(CH // 512):
                    nc.tensor.matmul(
                        ps[:, b_ * 512 : (b_ + 1) * 512], qT_a[:],
                        nmT_a[:, m0 + b_ * 512 : m0 + (b_ + 1) * 512],
                        start=True, stop=False)
                for b_ in range(CH // 512):
                    nc.tensor.matmul(
                        ps[:, b_ * 512 : (b_ + 1) * 512], qT_b[:],
                        nmT_b[:, m0 + b_ * 512 : m0 + (b_ + 1) * 512],
                        start=False, stop=True)
                s0 = (c * CH) % SEC
                nc.scalar.copy(sim[:, s0 : s0 + CH], ps[:])

            def screen_quarter(h, candv, candi, sim):
                nc.vector.max(out=candv[:, h * 8 : (h + 1) * 8], in_=sim[:])
                nc.vector.max_index(
                    out=candi[:],
                    in_max=candv[:, h * 8 : (h + 1) * 8],
                    in_values=sim[:],
                )

            def refine_gather(qt, candv, candif):
                # prune 32 quarter-candidates to top-16 and extract their
                # row ids (baseline-proven is_equal pattern, fp16 tiebreak
                # keeps values distinct)
                cvt = small.tile([128, NCAND], F32, tag="cvt")
                nc.vector.tensor_tensor(
                    out=cvt[:], in0=candv[:], in1=tb32[:],
                    op=mybir.AluOpType.add)
                pv1 = small.tile([128, 8], F32, tag="pv1")
                nc.vector.max(out=pv1[:], in_=cvt[:])
                cv2 = small.tile([128, NCAND], F32, tag="cv2")
                nc.vector.match_replace(
                    out=cv2[:], in_to_replace=pv1[:], in_values=cvt[:],
                    imm_value=NEG)
                pv2 = small.tile([128, 8], F32, tag="pv2")
                nc.vector.max(out=pv2[:], in_=cv2[:])
                pidx = small.tile([128, LP], F32, tag="pidx")
                msk = small.tile([128, NCAND], F32, tag="msk")
                mprod = small.tile([128, NCAND], F32, tag="mprod")
                for kk in range(LP):
                    pv = pv1 if kk < 8 else pv2
                    srcv = cvt if kk < 8 else cv2
                    nc.vector.tensor_scalar(
                        out=msk[:], in0=srcv[:],
                        scalar1=pv[:, kk % 8 : kk % 8 + 1], scalar2=None,
                        op0=mybir.AluOpType.is_equal)
                    nc.vector.tensor_tensor(
                        out=mprod[:], in0=msk[:], in1=candif[:],
                        op=mybir.AluOpType.mult)
                    nc.vector.tensor_reduce(
                        out=pidx[:, kk : kk + 1], in_=mprod[:],
                        axis=mybir.AxisListType.X, op=mybir.AluOpType.add)
                rows = small.tile([128, LP], U32, tag="rows")
                nc.vector.tensor_copy(rows[:], pidx[:])
                if DEBUG and qt == 0:
                    nc.sync.dma_start(dbg["rows"][:, :], rows[:])
                    nc.sync.dma_start(dbg["roff"][:, :], rows[:])
                for k in range(LP):
                    nc.gpsimd.indirect_dma_start(
                        out=G[:, k * D : (k + 1) * D],
                        out_offset=None,
                        in_=mem_in[:],
                        in_offset=IndirectOffsetOnAxis(
                            ap=rows[:, k : k + 1], axis=0))
                RV = small.tile([128, LP], F32, tag="RV")
                for k in range(LP):
                    nc.gpsimd.indirect_dma_start(
                        out=RV[:, k : k + 1],
                        out_offset=None,
                        in_=rinv_dram[:],
                        in_offset=IndirectOffsetOnAxis(
                            ap=rows[:, k : k + 1], axis=0))

                return RV

            def refine_finish(qt, qnorm, nqf, RV):
                q0 = qt * 128
                if DEBUG and qt == 0:
                    nc.sync.dma_start(dbg["G"][:, :], G[:])
                    nc.sync.dma_start(dbg["RV"][:, :], RV[:])
                # dots: gpsimd does the (G * nq) products in quarters, DVE
                # only the 3D reduces
                dots = small.tile([128, LP], F32, tag="dots")
                QW = 2  # candidates per sub-batch
                gq = prep1.tile([128, QW * D], F32, tag="gq")
                nq3 = nqf[:].rearrange("p (o d) -> p o d", o=1).to_broadcast(
                    [128, QW, D])
                for h in range(LP // QW):
                    nc.gpsimd.tensor_tensor(
                        out=gq[:].rearrange("p (k d) -> p k d", k=QW),
                        in0=G[:, h * QW * D : (h + 1) * QW * D].rearrange(
                            "p (k d) -> p k d", k=QW),
                        in1=nq3, op=mybir.AluOpType.mult)
                    nc.vector.tensor_reduce(
                        out=dots[:, h * QW : (h + 1) * QW],
                        in_=gq[:].rearrange("p (k d) -> p k d", k=QW),
                        axis=mybir.AxisListType.X, op=mybir.AluOpType.add)
                refined = small.tile([128, LP], F32, tag="refined")
                nc.vector.tensor_tensor(
                    out=refined[:], in0=dots[:], in1=RV[:],
                    op=mybir.AluOpType.mult)

                if DEBUG and qt == 0:
                    nc.sync.dma_start(dbg["dots"][:, :], dots[:])
                # self-match mask (exact, fp32)
                selfm = small.tile([128, LP], F32, tag="selfm")
                nc.vector.tensor_scalar(
                    out=selfm[:], in0=refined[:], scalar1=SELF_MATCH,
                    scalar2=NEG, op0=mybir.AluOpType.is_ge,
                    op1=mybir.AluOpType.mult)
                nc.vector.tensor_add(refined[:], refined[:], selfm[:])

                if DEBUG and qt == 0:
                    nc.sync.dma_start(dbg["refined"][:, :], refined[:])
                top8 = small.tile([128, 8], F32, tag="top8")
                nc.vector.max(out=top8[:], in_=refined[:])
                wmask = small.tile([128, LP], F32, tag="wmask")
                nc.vector.tensor_scalar(
                    out=wmask[:], in0=refined[:], scalar1=top8[:, 7:8],
                    scalar2=None, op0=mybir.AluOpType.is_ge)
                shift = small.tile([128, LP], F32, tag="shift")
                nc.vector.tensor_scalar(
                    out=shift[:], in0=refined[:], scalar1=top8[:, 0:1],
                    scalar2=None, op0=mybir.AluOpType.subtract)
                expv = small.tile([128, LP], F32, tag="expv")
                nc.scalar.activation(
                    expv[:], shift[:], mybir.ActivationFunctionType.Exp)
                wts = small.tile([128, LP], F32, tag="wts")
                nc.vector.tensor_tensor(
                    out=wts[:], in0=expv[:], in1=wmask[:],
                    op=mybir.AluOpType.mult)
                zsum = small.tile([128, 1], F32, tag="zsum")
                nc.vector.tensor_reduce(
                    out=zsum[:], in_=wts[:], axis=mybir.AxisListType.X,
                    op=mybir.AluOpType.add)
                zrin = small.tile([128, 1], F32, tag="zrin")
                nc.vector.reciprocal(zrin[:], zsum[:])
                nc.vector.tensor_scalar(
                    out=wts[:], in0=wts[:], scalar1=zrin[:, 0:1],
                    scalar2=None, op0=mybir.AluOpType.mult)

                if DEBUG and qt == 0:
                    nc.sync.dma_start(dbg["wts"][:, :], wts[:])
                # weighted sum on gpsimd: G *= w (in place), then reduce over k
                g3 = G[:].rearrange("p (k d) -> p k d", k=LP)
                w3 = wts[:].rearrange("p (k o) -> p k o", k=LP).to_broadcast(
                    [128, LP, D])
                nc.gpsimd.tensor_tensor(
                    out=g3, in0=g3, in1=w3, op=mybir.AluOpType.mult)
                acc = prep1.tile([128, D], F32, tag="acc")
                nc.vector.tensor_reduce(
                    out=acc[:],
                    in_=G[:].rearrange("p (k d) -> p d k", k=LP),
                    axis=mybir.AxisListType.X,
                    op=mybir.AluOpType.add)

                if DEBUG and qt == 0:
                    nc.sync.dma_start(dbg["acc"][:, :], acc[:])
                # renormalize to ||q||
                atr = prep1.tile([128, D], F32, tag="trash")
                an2 = small.tile([128, 1], F32, tag="an2")
                nc.vector.tensor_tensor(
                    out=atr[:], in0=acc[:], in1=acc[:], op=mybir.AluOpType.mult)
                nc.vector.tensor_reduce(
                    out=an2[:], in_=atr[:], axis=mybir.AxisListType.X,
                    op=mybir.AluOpType.add)
                an = small.tile([128, 1], F32, tag="an")
                nc.scalar.activation(
                    an[:], an2[:], mybir.ActivationFunctionType.Sqrt)
                nc.vector.tensor_scalar_max(an[:], an[:], EPS)
                arin = small.tile([128, 1], F32, tag="arin")
                nc.vector.reciprocal(arin[:], an[:])
                scl = small.tile([128, 1], F32, tag="scl")
                nc.vector.tensor_tensor(
                    out=scl[:], in0=arin[:], in1=qnorm[:],
                    op=mybir.AluOpType.mult)
                ot = prep1.tile([128, D], F32, tag="ot")
                nc.scalar.activation(
                    ot[:], acc[:], mybir.ActivationFunctionType.Copy,
                    scale=scl[:])
                nc.sync.dma_start(out[q0 : q0 + 128, :], ot[:])

            # ---------------- emission: prep chased by tile 0 -------------
            GPC = CH // GR  # groups per chunk = 4

            # software-pipelined emission: refine of tile t-1 is emitted
            # after the screens of tile t, so the scalar engine's evacs of
            # tile t are not queued behind tile t-1's refine-phase
            # activations, and the DVE alternates screens / refine work.
            # Prep groups are interleaved with tile-0 chunks so the bank
            # build is chased by the first tile.
            GPC = CH // GR  # groups per chunk

            NQRT = M // SEC  # quarters per tile
            CPQ = SEC // CH  # chunks per quarter

            def cand_convert(h, candv, candif, candi):
                # candidate col ids -> global f32 row ids (on gpsimd, off
                # the saturated DVE)
                nc.gpsimd.tensor_copy(
                    candif[:, h * 8 : (h + 1) * 8], candi[:])
                if h:
                    nc.gpsimd.tensor_scalar(
                        out=candif[:, h * 8 : (h + 1) * 8],
                        in0=candif[:, h * 8 : (h + 1) * 8],
                        scalar1=float(h * SEC), scalar2=None,
                        op0=mybir.AluOpType.add)

            # ---- phase 1: pure prep. Interleaving tiles with the bank
            # build measured SLOWER (the prep chain's scalar/DVE hops queue
            # behind tile evacs/screens in engine program order); with clean
            # queues the 64 groups pipeline at full depth.
            for g in range(NG):
                prep_group(g)
            pending = None

            # ---- phase 2: tiles 0..7, refine of tile t-1 interleaved after
            # quarter 1 so its scalar/gpsimd work overlaps the screens
            gathered = None
            for qt in range(0, NQT):
                qf, qnorm, nqf, qT_a, qT_b = query_prep(qt)
                candv = small.tile([128, NCAND], F16, tag="candv")
                candif = small.tile([128, NCAND], F32, tag="candif")
                for h in range(NQRT):
                    sim = secp.tile([128, SEC], F16, tag="sim")
                    for c in range(h * CPQ, (h + 1) * CPQ):
                        tile_chunk(c, qT_a, qT_b, sim)
                    candi = small.tile([128, 8], U16, tag="candi")
                    screen_quarter(h, candv, candi, sim)
                    cand_convert(h, candv, candif, candi)
                    if h == 1 and pending is not None:
                        pq, pqn, pnq, pcv, pcf = pending
                        rv = refine_gather(pq, pcv, pcf)
                        gathered = (pq, pqn, pnq, rv)
                        pending = None
                    if h == 3 and gathered is not None:
                        refine_finish(*gathered)
                        gathered = None
                pending = (qt, qnorm, nqf, candv, candif)
            pq, pqn, pnq, pcv, pcf = pending
            rv = refine_gather(pq, pcv, pcf)
            refine_finish(pq, pqn, pnq, rv)

    return nc


_CACHED_NC = None


def _get_nc():
    global _CACHED_NC
    if _CACHED_NC is None:
        _install_patches()
        _CACHED_NC = _build()
    return _CACHED_NC


def kernel(query, memory, k):
    query = np.ascontiguousarray(np.asarray(query, dtype=np.float32))
    memory = np.ascontiguousarray(np.asarray(memory, dtype=np.float32))
    k_val = int(np.asarray(k))
    assert query.shape == (B, D) and memory.shape == (M, D), (query.shape, memory.shape)
    assert k_val == K, f"kernel compiled for k={K}, got {k_val}"

    from concourse.bass_utils import run_bass_kernel_spmd

    nc = _get_nc()
    in_maps = [
        {"q": query[i * B_LOC : (i + 1) * B_LOC], "mem": memory}
        for i in range(N_CORES)
    ]
    res = run_bass_kernel_spmd(nc, in_maps, list(range(N_CORES)))
    return np.concatenate([res.results[i]["out"] for i in range(N_CORES)], axis=0)
